# revision 29
# baseline (speedup 1.0000x reference)
"""Bahdanau attention Trainium2 kernel.

reference:
  q = query @ Wa_w.T + Wa_b                # [B,1,H]
  k = keys @ Ua_w.T + Ua_b                 # [B,S,H]
  e = tanh(q + k)                          # [B,S,H]
  scores = e @ Va_w[0] + Va_b[0]           # [B,S]   (Va_b dropped: softmax-invariant)
  weights = softmax(scores)                # [B,1,S]
  context = weights @ keys                 # [B,1,H]

Sharding: data-parallel over batch B=32 across 8 cores (4 batches/core),
weights replicated (fed pre-transposed from host — layout prep only).
All PE matmuls in float32r (TF32-class rounding, ~1 cycle/row) except the
final context matmul (bf16). keys is transposed on-chip via PE
transpose-mode matmuls (fp32 has no DMA-transpose path).

Self-contained: builds the Bass program, compiles, runs on cores 0-7 via
run_bass_kernel_spmd, gathers shards.
"""
import sys
import numpy as np

for _p in ("/opt/trn_rl_repo", "/root/.axon_site/_ro/trn_rl_repo"):
    if _p not in sys.path:
        sys.path.append(_p)

import concourse.bacc as bacc
import concourse.mybir as mybir
from concourse.tile import TileContext
from concourse.masks import make_identity
from concourse import bass_utils

H = 1024
S = 2048
B = 32
NCORES = 8
BLOC = B // NCORES          # 4 batches per core
NCH = S // 512              # 4 s-chunks of 512
F32 = mybir.dt.float32
F32R = mybir.dt.float32r
BF16 = mybir.dt.bfloat16
ACT = mybir.ActivationFunctionType
AX = mybir.AxisListType


def build():
    nc = bacc.Bacc(
        "TRN2",
        target_bir_lowering=False,
        debug=False,
        enable_asserts=False,
        num_devices=1,
    )
    # host-prepared layouts (pure data movement):
    #   qTp     [128, 8*BLOC]  qTp[p, 4j+b] = query[b, 128j+p]
    #   ua_wT   [H, H]         Ua_w.T (contiguous)
    #   wa_wT   [H, H]         Wa_w.T
    #   bias_T  [128, 8]       bias_T[p, j] = (Wa_b+Ua_b)[128j+p]
    #   va_T    [128, 8]       va_T[p, j] = Va_w[0, 128j+p]
    qtp_d = nc.dram_tensor("qTp", [128, 8 * BLOC], F32, kind="ExternalInput")
    k_d = nc.dram_tensor("keys", [BLOC, S, H], F32, kind="ExternalInput")
    wat_d = nc.dram_tensor("wa_wT", [H, H], F32, kind="ExternalInput")
    uat_d = nc.dram_tensor("ua_wT", [H, H], F32, kind="ExternalInput")
    bt_d = nc.dram_tensor("bias_T", [128, 8], F32, kind="ExternalInput")
    vt_d = nc.dram_tensor("va_T", [128, 8], F32, kind="ExternalInput")
    ctx_d = nc.dram_tensor("ctx", [BLOC, H], F32, kind="ExternalOutput")
    wout_d = nc.dram_tensor("wout", [BLOC, S], F32, kind="ExternalOutput")

    with TileContext(nc) as tc:
        with (
            tc.tile_pool(name="const", bufs=1) as const,
            tc.tile_pool(name="stage", bufs=1) as stage,
            tc.tile_pool(name="main", bufs=2) as main,
            tc.tile_pool(name="small", bufs=1) as small,
            tc.tile_pool(name="t_ps", bufs=3, space="PSUM") as t_ps,
            tc.tile_pool(name="g_ps", bufs=2, space="PSUM") as g_ps,
            tc.tile_pool(name="va_ps", bufs=1, space="PSUM") as va_ps,
            tc.tile_pool(name="c_ps", bufs=2, space="PSUM") as c_ps,
        ):
            ident = const.tile([128, 128], F32, tag="ident")
            make_identity(nc, ident[:])

            # ---- constants from host layouts ----
            biasTt = const.tile([128, 8], F32, tag="biasTt")
            nc.sync.dma_start(biasTt[:], bt_d.ap())
            vstage = small.tile([128, 8], F32, tag="vstage")
            nc.sync.dma_start(vstage[:], vt_d.ap())
            vaTt = const.tile([128, 8], F32R, tag="vaTt")
            nc.vector.tensor_copy(vaTt[:], vstage[:])
            qstage = small.tile([128, 8 * BLOC], F32, tag="qstage")
            nc.sync.dma_start(qstage[:], qtp_d.ap())
            qTt = const.tile([128, 8 * BLOC], F32R, tag="qTt")
            nc.vector.tensor_copy(qTt[:], qstage[:])

            # ---- UaT[j] [128h, 1024o] f32r (DMA + round) ----
            UaT = []
            for j in range(8):
                us = stage.tile([128, H], F32, tag="knat", bufs=10, name=f"us{j}")
                nc.sync.dma_start(us[:], uat_d.ap()[j * 128:(j + 1) * 128, :])
                ut = const.tile([128, H], F32R, tag=f"uaT{j}", name=f"UaT{j}")
                nc.vector.tensor_copy(ut[:], us[:])
                UaT.append(ut)

            # ---- qb[b, o] = query @ Wa^T : lhsT = qT (tiny), rhs = WaT ----
            gq = [
                g_ps.tile([128, 512], F32, tag="g", name=f"gq{i}") for i in range(2)
            ]
            for h in range(8):
                ws = stage.tile([128, H], F32, tag="knat", bufs=10, name=f"ws{h}")
                nc.sync.dma_start(ws[:], wat_d.ap()[h * 128:(h + 1) * 128, :])
                wat = stage.tile([128, H], F32R, tag="waT", bufs=2, name=f"waT{h}")
                nc.vector.tensor_copy(wat[:], ws[:])
                for half in range(2):
                    nc.tensor.matmul(
                        gq[half][0:BLOC, :],
                        qTt[:, BLOC * h:BLOC * (h + 1)],
                        wat[:, half * 512:(half + 1) * 512],
                        start=(h == 0),
                        stop=(h == 7),
                    )
            qb = small.tile([BLOC, H], F32, tag="qb")
            for half in range(2):
                nc.vector.tensor_copy(
                    qb[:, half * 512:(half + 1) * 512], gq[half][0:BLOC, :]
                )
            # transpose qb -> qbT[o] [128, BLOC] f32, add bias during evict
            qbT = [
                const.tile([128, BLOC], F32, tag=f"qbT{o}", name=f"qbT{o}")
                for o in range(8)
            ]
            for o in range(8):
                t = t_ps.tile([128, 512], F32, tag="t")
                nc.tensor.transpose(
                    t[:, 0:BLOC], qb[:, o * 128:(o + 1) * 128],
                    ident[0:BLOC, 0:BLOC],
                )
                nc.scalar.activation(
                    qbT[o][:], t[:, 0:BLOC], ACT.Identity, bias=biasTt[:, o:o + 1]
                )

            # ---- main loop: chunks with pipelined epilogues ----
            # global chunk index q = 4*b + c; keys DMA + PE transposes for
            # chunk q+1 are woven between GEMM o-groups of chunk q so the PE
            # never stalls at a chunk boundary.
            kT_store = {}
            kbf_store = {}

            def preload(q):
                b, c = divmod(q, NCH)
                knat = []
                for si in range(4):
                    kn = stage.tile(
                        [128, H], F32, tag="knat", bufs=10, name=f"kn{q}_{si}"
                    )
                    r0 = c * 512 + si * 128
                    nc.sync.dma_start(kn[:], k_d.ap()[b, r0:r0 + 128, :])
                    knat.append(kn)
                return knat

            def transp_group(q, h, knat):
                t = t_ps.tile([128, 512], F32, tag="t")
                for si in range(4):
                    nc.tensor.matmul(
                        t[:, si * 128:(si + 1) * 128],
                        knat[si][:, h * 128:(h + 1) * 128],
                        ident[:],
                        is_transpose=True,
                        start=(si == 0),
                        stop=(si == 3),
                    )
                kt = main.tile([128, 512], F32R, tag=f"kT{h}", name=f"kT{q}_{h}")
                nc.vector.tensor_copy(kt[:], t[:])
                kT_store.setdefault(q, {})[h] = kt
                if h == 7:
                    # keys block already resident: cast to bf16 for the
                    # context matmul (replaces the HBM re-read + SWDGE cast)
                    b, c = divmod(q, NCH)
                    for si in range(4):
                        blk = c * 4 + si
                        kbf = main.tile(
                            [128, H], BF16, tag=f"kbf{blk}", bufs=1,
                            name=f"kbf{q}_{si}",
                        )
                        nc.vector.tensor_copy(kbf[:], knat[si][:])
                        kbf_store[(b, blk)] = kbf

            def gemm_chunk(q, sc, scm, weave, lag, mid=None):
                b, c = divmod(q, NCH)
                kT = kT_store.pop(q)
                va = va_ps.tile([1, 512], F32, tag="va")
                pending = []  # lag Va-dot so PE never waits on tanh
                def flush_one():
                    po, pe_ = pending.pop(0)
                    nc.tensor.matmul(
                        va[:], vaTt[:, po:po + 1], pe_[:],
                        start=(po == 0), stop=(po == 7),
                    )
                for o in range(8):
                    g = g_ps.tile([128, 512], F32, tag="g")
                    for h in range(8):
                        nc.tensor.matmul(
                            g[:],
                            UaT[h][:, o * 128:(o + 1) * 128],
                            kT[h][:],
                            start=(h == 0),
                            stop=(h == 7),
                        )
                    weave(o)
                    if mid is not None and o == 3:
                        mid()
                    e = main.tile([128, 512], F32R, tag="e", bufs=7)
                    nc.scalar.activation(
                        e[:], g[:], ACT.Tanh, bias=qbT[o][:, b:b + 1]
                    )
                    pending.append((o, e))
                    if len(pending) > lag:
                        flush_one()
                while pending:
                    flush_one()
                nc.vector.tensor_copy(sc[0:1, c * 512:(c + 1) * 512], va[:])
                nc.vector.reduce_max(
                    scm[0:1, c:c + 1], sc[0:1, c * 512:(c + 1) * 512], axis=AX.X
                )

            def epilogue(b, sc, scm):
                negm = small.tile([1, 1], F32, tag="negm", bufs=2)
                nc.vector.reduce_max(negm[:], scm[:], axis=AX.X, negate=True)
                u = small.tile([1, S], F32, tag="u", bufs=1)
                z = small.tile([1, 1], F32, tag="z", bufs=2)
                nc.scalar.activation(
                    u[:], sc[:], ACT.Exp, bias=negm[:], accum_out=z[:]
                )
                r = small.tile([1, 1], F32, tag="r", bufs=2)
                nc.vector.reciprocal(r[:], z[:])
                # w^T tiles from UNNORMALIZED u (softmax scale folded into the
                # context eviction) so the PE path does not wait on 1/Z.
                wT = []
                for tt in range(16):
                    t = t_ps.tile([128, 512], F32, tag="t")
                    nc.tensor.transpose(
                        t[:, 0:1],
                        u[0:1, tt * 128:(tt + 1) * 128],
                        ident[0:1, 0:1],
                    )
                    wt = small.tile(
                        [128, 1], BF16, tag=f"wT{tt}", bufs=2, name=f"wT{tt}"
                    )
                    nc.vector.tensor_copy(wt[:], t[:, 0:1])
                    wT.append(wt)
                cp = [
                    c_ps.tile([1, 512], F32, tag="c", name=f"cp{b}_{hh}")
                    for hh in range(2)
                ]
                for tt in range(16):
                    kb = kbf_store.pop((b, tt))
                    for half in range(2):
                        nc.tensor.matmul(
                            cp[half][:],
                            wT[tt][:],
                            kb[:, half * 512:(half + 1) * 512],
                            start=(tt == 0),
                            stop=(tt == 15),
                        )
                cs = small.tile([1, H], F32, tag="cs", bufs=2)
                for half in range(2):
                    nc.vector.tensor_scalar_mul(
                        cs[0:1, half * 512:(half + 1) * 512], cp[half][:], r[:]
                    )
                nc.sync.dma_start(ctx_d.ap()[b:b + 1, :], cs[:])
                w = small.tile([1, S], F32, tag="w", bufs=1)
                nc.vector.tensor_scalar_mul(w[:], u[:], r[:])
                nc.sync.dma_start(wout_d.ap()[b:b + 1, :], w[:])

            # startup: keys chunk 0 first so the PE has transpose work while
            # UaT/Wa stream in; then weights; then the pipelined chunk loop.
            knat0 = preload(0)
            for h in range(8):
                transp_group(0, h, knat0)
            UaT, qbT, vaTt = emit_setup()

            NQ = BLOC * NCH
            scs = {}
            scms = {}
            for q in range(NQ):
                b, c = divmod(q, NCH)
                if c == 0:
                    scs[b] = small.tile(
                        [1, S], F32, tag="sc", bufs=2, name=f"sc{b}"
                    )
                    scms[b] = small.tile(
                        [1, NCH], F32, tag="scm", bufs=2, name=f"scm{b}"
                    )
                knat_n = preload(q + 1) if q + 1 < NQ else None

                def weave(o, knat_n=knat_n, q=q):
                    if knat_n is not None:
                        transp_group(q + 1, o, knat_n)

                mid = None
                if c == 0 and b > 0:
                    bb = b - 1
                    mid = lambda bb=bb: epilogue(bb, scs[bb], scms[bb])
                gemm_chunk(q, scs[b], scms[b], weave, lag=5 if q == 0 else 3, mid=mid)
            epilogue(BLOC - 1, scs[BLOC - 1], scms[BLOC - 1])

    nc.compile()
    return nc


_NC_CACHE = {}


def _get_nc():
    if "nc" not in _NC_CACHE:
        _NC_CACHE["nc"] = build()
    return _NC_CACHE["nc"]


def make_in_maps(inputs):
    query = np.ascontiguousarray(np.asarray(inputs["query"], dtype=np.float32))
    keys = np.ascontiguousarray(np.asarray(inputs["keys"], dtype=np.float32))
    wa_w = np.asarray(inputs["Wa_w"], dtype=np.float32)
    wa_b = np.asarray(inputs["Wa_b"], dtype=np.float32).reshape(H)
    ua_w = np.asarray(inputs["Ua_w"], dtype=np.float32)
    ua_b = np.asarray(inputs["Ua_b"], dtype=np.float32).reshape(H)
    va_w = np.asarray(inputs["Va_w"], dtype=np.float32).reshape(H)

    wa_wT = np.ascontiguousarray(wa_w.T)
    ua_wT = np.ascontiguousarray(ua_w.T)
    bias_T = np.ascontiguousarray((wa_b + ua_b).reshape(8, 128).T)
    va_T = np.ascontiguousarray(va_w.reshape(8, 128).T)

    in_maps = []
    for cid in range(NCORES):
        b0 = cid * BLOC
        qs = query[b0:b0 + BLOC, 0, :]              # [BLOC, H]
        # qTp[p, 4j+b] = qs[b, 128j+p]
        qTp = np.ascontiguousarray(
            qs.reshape(BLOC, 8, 128).transpose(2, 1, 0).reshape(128, 8 * BLOC)
        )
        in_maps.append(
            {
                "qTp": qTp,
                "keys": np.ascontiguousarray(keys[b0:b0 + BLOC]),
                "wa_wT": wa_wT,
                "ua_wT": ua_wT,
                "bias_T": bias_T,
                "va_T": va_T,
            }
        )
    return in_maps


def kernel(**inputs):
    import time as _time

    nc = _get_nc()
    in_maps = make_in_maps(inputs)
    last_err = None
    for attempt in range(3):
        try:
            res = bass_utils.run_bass_kernel_spmd(
                nc, in_maps, core_ids=list(range(NCORES))
            )
            break
        except Exception as err:  # transient NRT device errors: retry
            last_err = err
            _time.sleep(5)
    else:
        raise last_err
    ctx = np.concatenate(
        [res.results[c]["ctx"] for c in range(NCORES)], axis=0
    ).reshape(B, 1, H)
    wout = np.concatenate(
        [res.results[c]["wout"] for c in range(NCORES)], axis=0
    ).reshape(B, 1, S)
    return (ctx, wout)


# revision 30
# speedup vs baseline: 1.0107x; 1.0107x over previous
"""Bahdanau attention Trainium2 kernel.

reference:
  q = query @ Wa_w.T + Wa_b                # [B,1,H]
  k = keys @ Ua_w.T + Ua_b                 # [B,S,H]
  e = tanh(q + k)                          # [B,S,H]
  scores = e @ Va_w[0] + Va_b[0]           # [B,S]   (Va_b dropped: softmax-invariant)
  weights = softmax(scores)                # [B,1,S]
  context = weights @ keys                 # [B,1,H]

Sharding: data-parallel over batch B=32 across 8 cores (4 batches/core),
weights replicated (fed pre-transposed from host — layout prep only).
All PE matmuls in float32r (TF32-class rounding, ~1 cycle/row) except the
final context matmul (bf16). keys is transposed on-chip via PE
transpose-mode matmuls (fp32 has no DMA-transpose path).

Self-contained: builds the Bass program, compiles, runs on cores 0-7 via
run_bass_kernel_spmd, gathers shards.
"""
import sys
import numpy as np

for _p in ("/opt/trn_rl_repo", "/root/.axon_site/_ro/trn_rl_repo"):
    if _p not in sys.path:
        sys.path.append(_p)

import concourse.bacc as bacc
import concourse.mybir as mybir
from concourse.tile import TileContext
from concourse.masks import make_identity
from concourse import bass_utils

H = 1024
S = 2048
B = 32
NCORES = 8
BLOC = B // NCORES          # 4 batches per core
NCH = S // 512              # 4 s-chunks of 512
F32 = mybir.dt.float32
F32R = mybir.dt.float32r
BF16 = mybir.dt.bfloat16
ACT = mybir.ActivationFunctionType
AX = mybir.AxisListType


def build():
    nc = bacc.Bacc(
        "TRN2",
        target_bir_lowering=False,
        debug=False,
        enable_asserts=False,
        num_devices=1,
    )
    # host-prepared layouts (pure data movement):
    #   qTp     [128, 8*BLOC]  qTp[p, 4j+b] = query[b, 128j+p]
    #   ua_wT   [H, H]         Ua_w.T (contiguous)
    #   wa_wT   [H, H]         Wa_w.T
    #   bias_T  [128, 8]       bias_T[p, j] = (Wa_b+Ua_b)[128j+p]
    #   va_T    [128, 8]       va_T[p, j] = Va_w[0, 128j+p]
    qtp_d = nc.dram_tensor("qTp", [128, 8 * BLOC], F32, kind="ExternalInput")
    k_d = nc.dram_tensor("keys", [BLOC, S, H], F32, kind="ExternalInput")
    wat_d = nc.dram_tensor("wa_wT", [H, H], F32, kind="ExternalInput")
    uat_d = nc.dram_tensor("ua_wT", [H, H], F32, kind="ExternalInput")
    bt_d = nc.dram_tensor("bias_T", [128, 8], F32, kind="ExternalInput")
    vt_d = nc.dram_tensor("va_T", [128, 8], F32, kind="ExternalInput")
    ctx_d = nc.dram_tensor("ctx", [BLOC, H], F32, kind="ExternalOutput")
    wout_d = nc.dram_tensor("wout", [BLOC, S], F32, kind="ExternalOutput")

    with TileContext(nc) as tc:
        with (
            tc.tile_pool(name="const", bufs=1) as const,
            tc.tile_pool(name="stage", bufs=1) as stage,
            tc.tile_pool(name="main", bufs=2) as main,
            tc.tile_pool(name="small", bufs=1) as small,
            tc.tile_pool(name="t_ps", bufs=3, space="PSUM") as t_ps,
            tc.tile_pool(name="g_ps", bufs=2, space="PSUM") as g_ps,
            tc.tile_pool(name="va_ps", bufs=1, space="PSUM") as va_ps,
            tc.tile_pool(name="c_ps", bufs=2, space="PSUM") as c_ps,
        ):
            ident = const.tile([128, 128], F32, tag="ident")
            make_identity(nc, ident[:])

            # ---- constants from host layouts ----
            biasTt = const.tile([128, 8], F32, tag="biasTt")
            nc.sync.dma_start(biasTt[:], bt_d.ap())
            vstage = small.tile([128, 8], F32, tag="vstage")
            nc.sync.dma_start(vstage[:], vt_d.ap())
            vaTt = const.tile([128, 8], F32R, tag="vaTt")
            nc.vector.tensor_copy(vaTt[:], vstage[:])
            qstage = small.tile([128, 8 * BLOC], F32, tag="qstage")
            nc.sync.dma_start(qstage[:], qtp_d.ap())
            qTt = const.tile([128, 8 * BLOC], F32R, tag="qTt")
            nc.vector.tensor_copy(qTt[:], qstage[:])

            # ---- UaT[j] [128h, 1024o] f32r (DMA + round) ----
            UaT = []
            for j in range(8):
                us = stage.tile([128, H], F32, tag="knat", bufs=10, name=f"us{j}")
                nc.sync.dma_start(us[:], uat_d.ap()[j * 128:(j + 1) * 128, :])
                ut = const.tile([128, H], F32R, tag=f"uaT{j}", name=f"UaT{j}")
                nc.vector.tensor_copy(ut[:], us[:])
                UaT.append(ut)

            # ---- qb[b, o] = query @ Wa^T : lhsT = qT (tiny), rhs = WaT ----
            gq = [
                g_ps.tile([128, 512], F32, tag="g", name=f"gq{i}") for i in range(2)
            ]
            for h in range(8):
                ws = stage.tile([128, H], F32, tag="knat", bufs=10, name=f"ws{h}")
                nc.sync.dma_start(ws[:], wat_d.ap()[h * 128:(h + 1) * 128, :])
                wat = stage.tile([128, H], F32R, tag="waT", bufs=2, name=f"waT{h}")
                nc.vector.tensor_copy(wat[:], ws[:])
                for half in range(2):
                    nc.tensor.matmul(
                        gq[half][0:BLOC, :],
                        qTt[:, BLOC * h:BLOC * (h + 1)],
                        wat[:, half * 512:(half + 1) * 512],
                        start=(h == 0),
                        stop=(h == 7),
                    )
            qb = small.tile([BLOC, H], F32, tag="qb")
            for half in range(2):
                nc.vector.tensor_copy(
                    qb[:, half * 512:(half + 1) * 512], gq[half][0:BLOC, :]
                )
            # transpose qb -> qbT[o] [128, BLOC] f32, add bias during evict
            qbT = [
                const.tile([128, BLOC], F32, tag=f"qbT{o}", name=f"qbT{o}")
                for o in range(8)
            ]
            for o in range(8):
                t = t_ps.tile([128, 512], F32, tag="t")
                nc.tensor.transpose(
                    t[:, 0:BLOC], qb[:, o * 128:(o + 1) * 128],
                    ident[0:BLOC, 0:BLOC],
                )
                nc.scalar.activation(
                    qbT[o][:], t[:, 0:BLOC], ACT.Identity, bias=biasTt[:, o:o + 1]
                )

            # ---- main loop: chunks with pipelined epilogues ----
            # global chunk index q = 4*b + c; keys DMA + PE transposes for
            # chunk q+1 are woven between GEMM o-groups of chunk q so the PE
            # never stalls at a chunk boundary.
            kT_store = {}
            kbf_store = {}

            def preload(q):
                b, c = divmod(q, NCH)
                knat = []
                for si in range(4):
                    kn = stage.tile(
                        [128, H], F32, tag="knat", bufs=10, name=f"kn{q}_{si}"
                    )
                    r0 = c * 512 + si * 128
                    nc.sync.dma_start(kn[:], k_d.ap()[b, r0:r0 + 128, :])
                    knat.append(kn)
                return knat

            def transp_group(q, h, knat):
                t = t_ps.tile([128, 512], F32, tag="t")
                for si in range(4):
                    nc.tensor.matmul(
                        t[:, si * 128:(si + 1) * 128],
                        knat[si][:, h * 128:(h + 1) * 128],
                        ident[:],
                        is_transpose=True,
                        start=(si == 0),
                        stop=(si == 3),
                    )
                kt = main.tile([128, 512], F32R, tag=f"kT{h}", name=f"kT{q}_{h}")
                nc.vector.tensor_copy(kt[:], t[:])
                kT_store.setdefault(q, {})[h] = kt
                if h == 7:
                    # keys block already resident: cast to bf16 for the
                    # context matmul (replaces the HBM re-read + SWDGE cast)
                    b, c = divmod(q, NCH)
                    for si in range(4):
                        blk = c * 4 + si
                        kbf = main.tile(
                            [128, H], BF16, tag=f"kbf{blk}", bufs=1,
                            name=f"kbf{q}_{si}",
                        )
                        nc.vector.tensor_copy(kbf[:], knat[si][:])
                        kbf_store[(b, blk)] = kbf

            def gemm_chunk(q, sc, scm, weave, lag, mid=None):
                b, c = divmod(q, NCH)
                kT = kT_store.pop(q)
                va = va_ps.tile([1, 512], F32, tag="va")
                pending = []  # lag Va-dot so PE never waits on tanh
                def flush_one():
                    po, pe_ = pending.pop(0)
                    nc.tensor.matmul(
                        va[:], vaTt[:, po:po + 1], pe_[:],
                        start=(po == 0), stop=(po == 7),
                    )
                for o in range(8):
                    g = g_ps.tile([128, 512], F32, tag="g")
                    for h in range(8):
                        nc.tensor.matmul(
                            g[:],
                            UaT[h][:, o * 128:(o + 1) * 128],
                            kT[h][:],
                            start=(h == 0),
                            stop=(h == 7),
                        )
                    weave(o)
                    if mid is not None and o == 3:
                        mid()
                    e = main.tile([128, 512], F32R, tag="e", bufs=7)
                    nc.scalar.activation(
                        e[:], g[:], ACT.Tanh, bias=qbT[o][:, b:b + 1]
                    )
                    pending.append((o, e))
                    if len(pending) > lag:
                        flush_one()
                while pending:
                    flush_one()
                nc.vector.tensor_copy(sc[0:1, c * 512:(c + 1) * 512], va[:])
                nc.vector.reduce_max(
                    scm[0:1, c:c + 1], sc[0:1, c * 512:(c + 1) * 512], axis=AX.X
                )

            def epilogue(b, sc, scm):
                negm = small.tile([1, 1], F32, tag="negm", bufs=2)
                nc.vector.reduce_max(negm[:], scm[:], axis=AX.X, negate=True)
                u = small.tile([1, S], F32, tag="u", bufs=1)
                z = small.tile([1, 1], F32, tag="z", bufs=2)
                nc.scalar.activation(
                    u[:], sc[:], ACT.Exp, bias=negm[:], accum_out=z[:]
                )
                r = small.tile([1, 1], F32, tag="r", bufs=2)
                nc.vector.reciprocal(r[:], z[:])
                # w^T tiles from UNNORMALIZED u (softmax scale folded into the
                # context eviction) so the PE path does not wait on 1/Z.
                wT = []
                for tt in range(16):
                    t = t_ps.tile([128, 512], F32, tag="t")
                    nc.tensor.transpose(
                        t[:, 0:1],
                        u[0:1, tt * 128:(tt + 1) * 128],
                        ident[0:1, 0:1],
                    )
                    wt = small.tile(
                        [128, 1], BF16, tag=f"wT{tt}", bufs=2, name=f"wT{tt}"
                    )
                    nc.vector.tensor_copy(wt[:], t[:, 0:1])
                    wT.append(wt)
                cp = [
                    c_ps.tile([1, 512], F32, tag="c", name=f"cp{b}_{hh}")
                    for hh in range(2)
                ]
                for tt in range(16):
                    kb = kbf_store.pop((b, tt))
                    for half in range(2):
                        nc.tensor.matmul(
                            cp[half][:],
                            wT[tt][:],
                            kb[:, half * 512:(half + 1) * 512],
                            start=(tt == 0),
                            stop=(tt == 15),
                        )
                cs = small.tile([1, H], F32, tag="cs", bufs=2)
                for half in range(2):
                    nc.vector.tensor_scalar_mul(
                        cs[0:1, half * 512:(half + 1) * 512], cp[half][:], r[:]
                    )
                nc.sync.dma_start(ctx_d.ap()[b:b + 1, :], cs[:])
                w = small.tile([1, S], F32, tag="w", bufs=1)
                nc.vector.tensor_scalar_mul(w[:], u[:], r[:])
                nc.sync.dma_start(wout_d.ap()[b:b + 1, :], w[:])

            # startup: keys chunk 0 first so the PE has transpose work while
            # UaT/Wa stream in; then weights; then the pipelined chunk loop.
            knat0 = preload(0)
            for h in range(8):
                transp_group(0, h, knat0)
            UaT, qbT, vaTt = emit_setup()

            NQ = BLOC * NCH
            scs = {}
            scms = {}
            for q in range(NQ):
                b, c = divmod(q, NCH)
                if c == 0:
                    scs[b] = small.tile(
                        [1, S], F32, tag="sc", bufs=2, name=f"sc{b}"
                    )
                    scms[b] = small.tile(
                        [1, NCH], F32, tag="scm", bufs=2, name=f"scm{b}"
                    )
                knat_n = preload(q + 1) if q + 1 < NQ else None

                def weave(o, knat_n=knat_n, q=q):
                    # coarser weave: two transpose groups every other o-group
                    # keeps GEMM bursts 16 MMs long (better PE pipelining)
                    if knat_n is not None and o % 2 == 1:
                        transp_group(q + 1, o - 1, knat_n)
                        transp_group(q + 1, o, knat_n)

                mid = None
                if c == 0 and b > 0:
                    bb = b - 1
                    mid = lambda bb=bb: epilogue(bb, scs[bb], scms[bb])
                gemm_chunk(q, scs[b], scms[b], weave, lag=5 if q == 0 else 3, mid=mid)
            epilogue(BLOC - 1, scs[BLOC - 1], scms[BLOC - 1])

    nc.compile()
    return nc


_NC_CACHE = {}


def _get_nc():
    if "nc" not in _NC_CACHE:
        _NC_CACHE["nc"] = build()
    return _NC_CACHE["nc"]


def make_in_maps(inputs):
    query = np.ascontiguousarray(np.asarray(inputs["query"], dtype=np.float32))
    keys = np.ascontiguousarray(np.asarray(inputs["keys"], dtype=np.float32))
    wa_w = np.asarray(inputs["Wa_w"], dtype=np.float32)
    wa_b = np.asarray(inputs["Wa_b"], dtype=np.float32).reshape(H)
    ua_w = np.asarray(inputs["Ua_w"], dtype=np.float32)
    ua_b = np.asarray(inputs["Ua_b"], dtype=np.float32).reshape(H)
    va_w = np.asarray(inputs["Va_w"], dtype=np.float32).reshape(H)

    wa_wT = np.ascontiguousarray(wa_w.T)
    ua_wT = np.ascontiguousarray(ua_w.T)
    bias_T = np.ascontiguousarray((wa_b + ua_b).reshape(8, 128).T)
    va_T = np.ascontiguousarray(va_w.reshape(8, 128).T)

    in_maps = []
    for cid in range(NCORES):
        b0 = cid * BLOC
        qs = query[b0:b0 + BLOC, 0, :]              # [BLOC, H]
        # qTp[p, 4j+b] = qs[b, 128j+p]
        qTp = np.ascontiguousarray(
            qs.reshape(BLOC, 8, 128).transpose(2, 1, 0).reshape(128, 8 * BLOC)
        )
        in_maps.append(
            {
                "qTp": qTp,
                "keys": np.ascontiguousarray(keys[b0:b0 + BLOC]),
                "wa_wT": wa_wT,
                "ua_wT": ua_wT,
                "bias_T": bias_T,
                "va_T": va_T,
            }
        )
    return in_maps


def kernel(**inputs):
    import time as _time

    nc = _get_nc()
    in_maps = make_in_maps(inputs)
    last_err = None
    for attempt in range(3):
        try:
            res = bass_utils.run_bass_kernel_spmd(
                nc, in_maps, core_ids=list(range(NCORES))
            )
            break
        except Exception as err:  # transient NRT device errors: retry
            last_err = err
            _time.sleep(5)
    else:
        raise last_err
    ctx = np.concatenate(
        [res.results[c]["ctx"] for c in range(NCORES)], axis=0
    ).reshape(B, 1, H)
    wout = np.concatenate(
        [res.results[c]["wout"] for c in range(NCORES)], axis=0
    ).reshape(B, 1, S)
    return (ctx, wout)
